# revision 15
# baseline (speedup 1.0000x reference)
"""Trainium2 Bass kernel for BitmapGFN.sample (argmax, no terminate).

Problem: B=4096 batch rows each run a 64-step sequential rollout. Each step:
MLP(concat(x, state)) -> 129 logits; masked argmax over 64 forward bit-logits
picks a bit; log-probs of forward/backward/terminate policies are recorded;
the chosen bit is set in the state. Outputs: states [B,65,64], logpf [B,64],
logpb [B,64], logpt [B,65].

Sharding: pure data parallel, batch/8 = 512 rows per core, weights replicated.

Kernel layout (per core, batch 512 split into 2 halves x 256 for pipelining):
- Activations feature-major [feat_part, batch_free]; x@W1[:512]+b1 precomputed
  once (a1_0); per step only W1[512:].T @ state is computed (K=64).
- L2 feature-major: lhsT=W2 k/m tiles, rhs=h1.
- L3 batch-major: lhsT=h2 chunk (128 batch cols), rhs=W3 padded to 256 ->
  logits land [batch, 129] so softmax/argmax reductions run on the free axis.
- b3 folded in via an extra K=1 matmul row (ones lhsT, b3 rhs).
- argmax via reduce_max + is_equal -> one-hot; transposed (PE) to feature-major
  to update the feature-major state.
- Matmuls in fp32r. KSPLIT=1 (default) uses the 3-pass hi/lo split
  (hi = 10 explicit mantissa bits host-side; device fp32r keeps 11), giving
  fp32-grade accuracy at 3x the PE cost of 1-pass fp32r.
"""
import os
import numpy as np
import concourse.bacc as bacc
import concourse.mybir as mybir
from concourse.tile import TileContext

B, DIN, NB, DH = 4096, 512, 64, 1024
NCORES = 8
BC = B // NCORES          # 512 rows per core
NH = BC // 2              # 256 per half (fp32r needs moving dim >= 256)
NCH = BC // 128           # 4 chunks of 128 batch rows
W3P = 256                 # W3 free dim padded 129 -> 256
KT2 = DH // 128           # 8 k-tiles for DH
NEGF = -1.0e8

STEPS = int(os.environ.get("KSTEPS", str(NB)))
SPLIT = os.environ.get("KSPLIT", "1") == "1"
TRACE = os.environ.get("KTRACE", "0") == "1"

f32 = mybir.dt.float32
f32r = mybir.dt.float32r
A = mybir.AluOpType
AF = mybir.ActivationFunctionType
AX = mybir.AxisListType


def _trunc12(a):
    """Device fp32r rounding: truncate the low 12 mantissa bits."""
    u = np.ascontiguousarray(a, np.float32).view(np.uint32) & np.uint32(0xFFFFF000)
    return u.view(np.float32)


def _round_keep10(a):
    """Round fp32 to 10 explicit mantissa bits (RTN). Device fp32r keeps 11
    (truncates low 12), so these values pass through fp32r exactly."""
    u = np.ascontiguousarray(a, np.float32).view(np.uint32).astype(np.uint64)
    u = ((u + (1 << 12)) & np.uint64(0xFFFFE000)).astype(np.uint32)
    return u.view(np.float32)


def _split_w(a):
    hi = _round_keep10(a)
    lo = _trunc12((np.asarray(a, np.float64) - hi.astype(np.float64)).astype(np.float32))
    return hi, lo


_PINNED_ACT_SET = "natural_log_exp_and_others"
_orig_get_act_tables = None


def _pin_act_tables():
    """Make every activation resolve to the one table set containing
    relu/exp/ln/copy/identity, so the kernel loads the ACT table once
    instead of on every relu<->exp<->ln transition (~1.3us each)."""
    global _orig_get_act_tables
    import concourse.hw_specs as hw_specs
    if _orig_get_act_tables is None:
        _orig_get_act_tables = hw_specs.get_activation_tables

    def patched(module_arch):
        tables = dict(_orig_get_act_tables(module_arch))
        assert _PINNED_ACT_SET in tables, sorted(tables)
        return {
            name: (fns if name == _PINNED_ACT_SET else set())
            for name, fns in tables.items()
        }

    bacc.get_activation_tables = patched


def _build(steps: int, split: bool, b3nz: bool = True):
    _pin_act_tables()
    nc = bacc.Bacc("TRN2", target_bir_lowering=False)

    # ---- DRAM I/O ----
    xrh_d = nc.dram_tensor("xrh", [DIN, BC], f32r, kind="ExternalInput")
    xrl_d = nc.dram_tensor("xrl", [DIN, BC], f32r, kind="ExternalInput")
    w1xh_d = nc.dram_tensor("w1xh", [DIN, DH], f32r, kind="ExternalInput")
    w1xl_d = nc.dram_tensor("w1xl", [DIN, DH], f32r, kind="ExternalInput")
    w1sh_d = nc.dram_tensor("w1sh", [NB, DH], f32r, kind="ExternalInput")
    w1sl_d = nc.dram_tensor("w1sl", [NB, DH], f32r, kind="ExternalInput")
    w2h_d = nc.dram_tensor("w2h", [DH, DH], f32r, kind="ExternalInput")
    w2l_d = nc.dram_tensor("w2l", [DH, DH], f32r, kind="ExternalInput")
    w3h_d = nc.dram_tensor("w3h", [DH, W3P], f32r, kind="ExternalInput")
    w3l_d = nc.dram_tensor("w3l", [DH, W3P], f32r, kind="ExternalInput")
    b1_d = nc.dram_tensor("b1c", [128, KT2], f32, kind="ExternalInput")
    b2_d = nc.dram_tensor("b2c", [128, KT2], f32, kind="ExternalInput")
    b3_d = nc.dram_tensor("b3row", [1, W3P], f32, kind="ExternalInput")
    ident_d = nc.dram_tensor("ident", [128, 128], f32, kind="ExternalInput")

    states_d = nc.dram_tensor("states_o", [BC, (steps + 1) * NB], f32, kind="ExternalOutput")
    logpf_d = nc.dram_tensor("logpf_o", [BC, steps], f32, kind="ExternalOutput")
    logpb_d = nc.dram_tensor("logpb_o", [BC, steps], f32, kind="ExternalOutput")
    logpt_d = nc.dram_tensor("logpt_o", [BC, steps + 1], f32, kind="ExternalOutput")

    import contextlib
    with TileContext(nc) as tc, contextlib.ExitStack() as ctx:
        # ---------- persistent SBUF ----------
        persist = ctx.enter_context(tc.tile_pool(name="persist", bufs=1))

        def single(shape, dtype, name, pool=None):
            return (pool or persist).tile(shape, dtype, name=name, tag=name)

        a1_0 = [single([128, BC], f32, f"a1_0_{m}") for m in range(KT2)]
        # split: [W1sh; W1sl] packed rows 0-63/64-127, one K=128 matmul vs two K=64
        w1sp = single([2 * NB if split else NB, DH], f32r, "w1sp_t")
        b2t = single([128, KT2], f32, "b2t")
        b3row = single([1, W3P], f32r, "b3row_t")
        onesrow = single([1, 128], f32r, "onesrow")
        ident = single([128, 128], f32, "ident_t")

        SFM = 2 * NB if split else NB   # state duplicated across both K halves
        state_fm = [single([SFM, NH], f32r, f"state_fm_{h}") for h in range(2)]
        state_ext = [single([128, NB + 1], f32, f"state_ext_{c}") for c in range(NCH)]
        oh_buf = [[single([128, NB], f32, f"oh_{c}_{j}") for j in range(2)] for c in range(NCH)]
        oh_fm = [single([SFM, NH], f32r, f"oh_fm_{h}") for h in range(2)]
        lpf_acc = [single([128, steps], f32, f"lpf_{c}") for c in range(NCH)]
        lpb_acc = [single([128, steps], f32, f"lpb_{c}") for c in range(NCH)]
        lpt_acc = [single([128, steps], f32, f"lpt_{c}") for c in range(NCH)]

        # ---------- init ----------
        nc.sync.dma_start(b2t[:], b2_d[:])
        nc.sync.dma_start(ident[:], ident_d[:])
        # memset doesn't support fp32r; zero/one f32 scratch + DVE copy (rounds)
        zsrc = single([SFM, NH], f32, "zsrc")
        nc.gpsimd.memset(zsrc[:], 0.0)
        onesrc = single([1, 128], f32, "onesrc")
        nc.gpsimd.memset(onesrc[:], 1.0)
        for h in range(2):
            nc.vector.tensor_copy(state_fm[h][:], zsrc[:])
            nc.vector.tensor_copy(oh_fm[h][:], zsrc[:])
        for c in range(NCH):
            nc.gpsimd.memset(state_ext[c][:, :NB], 0.0)
            nc.gpsimd.memset(state_ext[c][:, NB:NB + 1], 1.0)
            nc.gpsimd.memset(oh_buf[c][0][:], 0.0)
            nc.gpsimd.memset(oh_buf[c][1][:], 0.0)
        nc.vector.tensor_copy(onesrow[:], onesrc[:])

        # b3 row -> fp32r via DVE (b3 is arbitrary fp32; DVE copy rounds it)
        with tc.tile_pool(name="ldst", bufs=2) as ldp:
            b3s = ldp.tile([1, W3P], f32, name="b3s")
            nc.sync.dma_start(b3s[:], b3_d[:])
            nc.vector.tensor_copy(b3row[:], b3s[:])

        # host-pre-rounded weights are exactly fp32r-representable: DMA direct
        nc.sync.dma_start(w1sp[:NB, :], w1sh_d[:])
        if split:
            nc.sync.dma_start(w1sp[NB:, :], w1sl_d[:])

        # ---------- precompute a1_0 = x @ W1[:DIN] + b1 (split-accurate) ----------
        with tc.tile_pool(name="pre", bufs=1) as prep, \
             tc.tile_pool(name="preps", bufs=2, space="PSUM") as preps:
            KTX = DIN // 128
            xr = [prep.tile([128, BC], f32r, name=f"xr_{k}", tag=f"xr_{k}") for k in range(KTX)]
            xl = [prep.tile([128, BC], f32r, name=f"xl_{k}", tag=f"xl_{k}") for k in range(KTX)]
            wxh = [prep.tile([128, DH], f32r, name=f"wxh_{k}", tag=f"wxh_{k}") for k in range(KTX)]
            wxl = [prep.tile([128, DH], f32r, name=f"wxl_{k}", tag=f"wxl_{k}") for k in range(KTX)]
            b1s = prep.tile([128, KT2], f32, name="b1s", tag="b1s")
            nc.sync.dma_start(b1s[:], b1_d[:])
            for k in range(KTX):
                nc.sync.dma_start(xr[k][:], xrh_d[k * 128:(k + 1) * 128, :])
                nc.sync.dma_start(xl[k][:], xrl_d[k * 128:(k + 1) * 128, :])
                nc.sync.dma_start(wxh[k][:], w1xh_d[k * 128:(k + 1) * 128, :])
                nc.sync.dma_start(wxl[k][:], w1xl_d[k * 128:(k + 1) * 128, :])
            for m in range(KT2):
                ps = preps.tile([128, BC], f32, name="ps_pre", tag="ps_pre")
                first = True
                for k in range(KTX):
                    ms = slice(m * 128, (m + 1) * 128)
                    nc.tensor.matmul(ps[:], wxh[k][:, ms], xr[k][:], start=first, stop=False)
                    first = False
                    nc.tensor.matmul(ps[:], wxh[k][:, ms], xl[k][:], start=False, stop=False)
                    nc.tensor.matmul(ps[:], wxl[k][:, ms], xr[k][:], start=False,
                                     stop=(k == KTX - 1))
                nc.vector.tensor_scalar(
                    out=a1_0[m][:], in0=ps[:], scalar1=b1s[:, m:m + 1], scalar2=None,
                    op0=A.add)

        # ---------- big persistent tensors (after precompute frees its pool) ----------
        persist2 = ctx.enter_context(tc.tile_pool(name="persist2", bufs=1))
        w2h = [single([128, DH], f32r, f"w2h_{k}", persist2) for k in range(KT2)]
        w2l = [single([128, DH], f32r, f"w2l_{k}", persist2) for k in range(KT2)] if split else None
        w3h = [single([128, W3P], f32r, f"w3h_{k}", persist2) for k in range(KT2)]
        w3l = [single([128, W3P], f32r, f"w3l_{k}", persist2) for k in range(KT2)] if split else None
        h1r = [[single([128, NH], f32r, f"h1r_{h}_{k}", persist2) for k in range(KT2)] for h in range(2)]
        h2r = [[single([128, NH], f32r, f"h2r_{h}_{k}", persist2) for k in range(KT2)] for h in range(2)]
        h1l = h2l = None
        if split:
            h1l = [[single([128, NH], f32r, f"h1l_{h}_{k}", persist2) for k in range(KT2)] for h in range(2)]
            h2l = [[single([128, NH], f32r, f"h2l_{h}_{k}", persist2) for k in range(KT2)] for h in range(2)]

        for k in range(KT2):
            nc.sync.dma_start(w2h[k][:], w2h_d[k * 128:(k + 1) * 128, :])
            nc.sync.dma_start(w3h[k][:], w3h_d[k * 128:(k + 1) * 128, :])
            if split:
                nc.sync.dma_start(w2l[k][:], w2l_d[k * 128:(k + 1) * 128, :])
                nc.sync.dma_start(w3l[k][:], w3l_d[k * 128:(k + 1) * 128, :])

        # ---------- main rollout ----------
        with tc.tile_pool(name="ps_l1", bufs=2, space="PSUM") as ps_l1_p, \
             tc.tile_pool(name="ps_l2", bufs=2, space="PSUM") as ps_l2_p, \
             tc.tile_pool(name="ps_l3", bufs=2, space="PSUM") as ps_l3_p, \
             tc.tile_pool(name="ps_tr", bufs=2, space="PSUM") as ps_tr_p, \
             tc.tile_pool(name="sc", bufs=3) as scp, \
             tc.tile_pool(name="sc1", bufs=4) as scp1:

            pending = [[], []]  # per half: [(chunk_slice, oh_tile)] from prev step
            for i in range(steps + 1):
                fin = (i == steps)
                for h in range(2):
                    hs = slice(h * NH, (h + 1) * NH)
                    # deferred from step i-1: one-hot -> feature-major, state update.
                    # Emitted here so the PE transposes sit behind this half's
                    # next matmul batch instead of stalling the PE queue on the
                    # sampling chain.
                    if pending[h]:
                        for cs_p, oh_p in pending[h]:
                            # one-hot -> feature-major; under split, duplicate into
                            # partitions 64-127 via col tile_position so the packed
                            # K=128 L1 matmul sees [state; state]
                            pt = ps_tr_p.tile([NB, 128], f32, name="pt", tag="pt")
                            nc.tensor.transpose(pt[:], oh_p[:], ident[:])
                            nc.vector.tensor_copy(oh_fm[h][:NB, cs_p], pt[:])
                            if split:
                                # duplicate into partitions 64-127 (partition
                                # shift -> DMA; DVE lanes can't cross partitions)
                                nc.sync.dma_start(oh_fm[h][NB:, cs_p],
                                                  oh_fm[h][:NB, cs_p])
                        nc.vector.tensor_add(
                            state_fm[h][:], state_fm[h][:].bitcast(f32),
                            oh_fm[h][:].bitcast(f32))
                        pending[h] = []
                    # ---- L1: a1 = a1_0 + W1s.T @ state_fm ; h1 = relu ----
                    for m in range(KT2):
                        ms = slice(m * 128, (m + 1) * 128)
                        ps = ps_l1_p.tile([128, NH], f32, name="ps1", tag="ps1")
                        nc.tensor.matmul(ps[:], w1sp[:, ms], state_fm[h][:],
                                         start=True, stop=True)
                        a1c = scp.tile([128, NH], f32, name="a1c", tag="a1c")
                        nc.vector.tensor_add(a1c[:], ps[:], a1_0[m][:, hs])
                        nc.scalar.activation(h1r[h][m][:], a1c[:], AF.Relu)
                        if split:
                            h1f = scp.tile([128, NH], f32, name="h1f", tag="h1f")
                            nc.scalar.activation(h1f[:], a1c[:], AF.Relu)
                            nc.vector.scalar_tensor_tensor(
                                h1l[h][m][:], h1r[h][m][:].bitcast(f32), -1.0, h1f[:],
                                A.mult, A.add)
                    # ---- L2 ----
                    for m in range(KT2):
                        ms = slice(m * 128, (m + 1) * 128)
                        ps = ps_l2_p.tile([128, NH], f32, name="ps2", tag="ps2")
                        for k in range(KT2):
                            last = (k == KT2 - 1)
                            nc.tensor.matmul(ps[:], w2h[k][:, ms], h1r[h][k][:],
                                             start=(k == 0), stop=last and not split)
                            if split:
                                nc.tensor.matmul(ps[:], w2h[k][:, ms], h1l[h][k][:],
                                                 start=False, stop=False)
                                nc.tensor.matmul(ps[:], w2l[k][:, ms], h1r[h][k][:],
                                                 start=False, stop=last)
                        nc.scalar.activation(h2r[h][m][:], ps[:], AF.Relu,
                                             bias=b2t[:, m:m + 1])
                        if split:
                            h2f = scp.tile([128, NH], f32, name="h2f", tag="h2f")
                            nc.scalar.activation(h2f[:], ps[:], AF.Relu,
                                                 bias=b2t[:, m:m + 1])
                            nc.vector.scalar_tensor_tensor(
                                h2l[h][m][:], h2r[h][m][:].bitcast(f32), -1.0, h2f[:],
                                A.mult, A.add)
                    # ---- L3 + sampling per chunk ----
                    for cc in range(2):
                        c = 2 * h + cc
                        cs = slice(cc * 128, (cc + 1) * 128)      # within-half cols
                        ps = ps_l3_p.tile([128, W3P], f32, name="ps3", tag="ps3")
                        if b3nz:
                            nc.tensor.matmul(ps[:], onesrow[:], b3row[:],
                                             start=True, stop=False)
                        for k in range(KT2):
                            last = (k == KT2 - 1)
                            nc.tensor.matmul(ps[:], h2r[h][k][:, cs], w3h[k][:],
                                             start=(k == 0 and not b3nz),
                                             stop=last and not split)
                            if split:
                                nc.tensor.matmul(ps[:], h2l[h][k][:, cs], w3h[k][:],
                                                 start=False, stop=False)
                                nc.tensor.matmul(ps[:], h2r[h][k][:, cs], w3l[k][:],
                                                 start=False, stop=last)
                        st = state_ext[c]
                        # backward policy (skip at i=0; the only part at i=steps)
                        if i > 0:
                            bmask = scp1.tile([128, NB], f32, name="bmask", tag="bmask")
                            nc.vector.tensor_scalar(
                                out=bmask[:], in0=st[:, :NB], scalar1=1.0e8,
                                scalar2=NEGF, op0=A.mult, op1=A.add)
                            mb = scp1.tile([128, NB], f32, name="mb", tag="mb")
                            nc.vector.tensor_add(mb[:], ps[:, NB + 1:2 * NB + 1], bmask[:])
                            m2 = scp1.tile([128, 1], f32, name="m2", tag="m2")
                            nc.vector.reduce_max(m2[:], mb[:], axis=AX.X)
                            nm2 = scp1.tile([128, 1], f32, name="nm2", tag="nm2")
                            nc.vector.tensor_scalar_mul(nm2[:], m2[:], -1.0)
                            e2 = scp1.tile([128, NB], f32, name="e2", tag="e2")
                            s2 = scp1.tile([128, 1], f32, name="s2", tag="s2")
                            nc.scalar.activation(e2[:], mb[:], AF.Exp, bias=nm2[:],
                                                 accum_out=s2[:])
                            ln2 = scp1.tile([128, 1], f32, name="ln2", tag="ln2")
                            nc.scalar.activation(ln2[:], s2[:], AF.Ln)
                            ttscr = scp1.tile([128, NB], f32, name="ttscr", tag="ttscr")
                            gb = scp1.tile([128, 1], f32, name="gb", tag="gb")
                            nc.vector.tensor_mul(
                                ttscr[:], ps[:, NB + 1:2 * NB + 1],
                                oh_buf[c][(i + 1) % 2][:])
                            nc.vector.reduce_sum(gb[:], ttscr[:], axis=AX.X)
                            t0 = scp1.tile([128, 1], f32, name="t0", tag="t0")
                            nc.vector.scalar_tensor_tensor(
                                t0[:], ln2[:], -1.0, gb[:], A.mult, A.add)
                            nc.vector.tensor_add(lpb_acc[c][:, i - 1:i], t0[:], nm2[:])
                        if fin:
                            continue
                        # forward policy + argmax
                        mf = scp1.tile([128, NB + 1], f32, name="mf", tag="mf")
                        nc.vector.scalar_tensor_tensor(
                            mf[:], st[:], NEGF, ps[:, :NB + 1], A.mult, A.add)
                        m1 = scp1.tile([128, 1], f32, name="m1", tag="m1")
                        nc.vector.reduce_max(m1[:], mf[:, :NB], axis=AX.X)
                        nm1 = scp1.tile([128, 1], f32, name="nm1", tag="nm1")
                        nc.vector.tensor_scalar_mul(nm1[:], m1[:], -1.0)
                        oh = oh_buf[c][i % 2]
                        nc.vector.tensor_scalar(
                            out=oh[:], in0=mf[:, :NB], scalar1=m1[:], scalar2=None,
                            op0=A.is_equal)
                        e1 = scp1.tile([128, NB + 1], f32, name="e1", tag="e1")
                        s1 = scp1.tile([128, 1], f32, name="s1", tag="s1")
                        nc.scalar.activation(e1[:], mf[:], AF.Exp, bias=nm1[:],
                                             accum_out=s1[:])
                        ln1 = scp1.tile([128, 1], f32, name="ln1", tag="ln1")
                        nc.scalar.activation(ln1[:], s1[:], AF.Ln)
                        # logpf = -ln(s1); logpt = -1e8 - m1 - ln(s1)
                        nc.vector.tensor_scalar_mul(lpf_acc[c][:, i:i + 1], ln1[:], -1.0)
                        t1 = scp1.tile([128, 1], f32, name="t1", tag="t1")
                        nc.vector.scalar_tensor_tensor(
                            t1[:], ln1[:], -1.0, nm1[:], A.mult, A.add)
                        nc.vector.tensor_scalar_add(lpt_acc[c][:, i:i + 1], t1[:], NEGF)
                        # state update + DMA out; FM conversion deferred
                        nc.vector.tensor_add(st[:, :NB], st[:, :NB], oh[:])
                        nc.sync.dma_start(
                            states_d[c * 128:(c + 1) * 128,
                                     (i + 1) * NB:(i + 2) * NB], st[:, :NB])
                        pending[h].append((cs, oh))

            # ---------- final DMA of log-prob accumulators ----------
            for c in range(NCH):
                rs = slice(c * 128, (c + 1) * 128)
                nc.sync.dma_start(logpf_d[rs, :], lpf_acc[c][:])
                nc.sync.dma_start(logpb_d[rs, :], lpb_acc[c][:])
                nc.sync.dma_start(logpt_d[rs, :steps], lpt_acc[c][:])

    nc.finalize()
    return nc


_RUN_CACHE = {}


def _run_spmd(nc, in_maps):
    """Execute the finalized Bass module on NCORES cores via PJRT shard_map.

    Same lowering as concourse.bass2jax.run_bass_via_pjrt but without output
    donation (donated zero-buffer shipping was flaky over the axon tunnel) and
    with the jitted executable cached for repeat calls. Outputs this kernel
    doesn't fully write (states plane 0, logpt last col) are host-filled by
    kernel().
    """
    import jax
    from jax.sharding import Mesh, PartitionSpec, NamedSharding
    from jax.experimental.shard_map import shard_map
    from concourse.bass2jax import (
        _bass_exec_p, partition_id_tensor, install_neuronx_cc_hook)

    key = id(nc)
    if key not in _RUN_CACHE:
        install_neuronx_cc_hook()
        partition_name = (nc.partition_id_tensor.name
                          if nc.partition_id_tensor else None)
        in_names, out_names, out_avals = [], [], []
        for alloc in nc.m.functions[0].allocations:
            if not isinstance(alloc, mybir.MemoryLocationSet):
                continue
            name = alloc.memorylocations[0].name
            if alloc.kind == "ExternalInput":
                if name != partition_name:
                    in_names.append(name)
            elif alloc.kind == "ExternalOutput":
                out_names.append(name)
                out_avals.append(jax.core.ShapedArray(
                    tuple(alloc.tensor_shape), mybir.dt.np(alloc.dtype)))
        all_in = in_names + out_names + ([partition_name] if partition_name else [])

        def _body(*args):
            operands = list(args)
            if partition_name is not None:
                operands.append(partition_id_tensor())
            return tuple(_bass_exec_p.bind(
                *operands, out_avals=tuple(out_avals), in_names=tuple(all_in),
                out_names=tuple(out_names), lowering_input_output_aliases=(),
                sim_require_finite=True, sim_require_nnan=True, nc=nc))

        devices = jax.devices()[:NCORES]
        mesh = Mesh(np.asarray(devices), ("core",))
        nio = len(in_names) + len(out_names)
        sharded = jax.jit(
            shard_map(_body, mesh=mesh,
                      in_specs=(PartitionSpec("core"),) * nio,
                      out_specs=(PartitionSpec("core"),) * len(out_names),
                      check_rep=False),
            keep_unused=True)
        sh = NamedSharding(mesh, PartitionSpec("core"))
        _RUN_CACHE[key] = (sharded, sh, in_names, out_names, out_avals)

    sharded, sh, in_names, out_names, out_avals = _RUN_CACHE[key]
    import jax
    concat_in = [
        np.concatenate([np.asarray(in_maps[c][nm]) for c in range(NCORES)], 0)
        for nm in in_names
    ]
    concat_zeros = [
        np.zeros((NCORES * a.shape[0], *a.shape[1:]), a.dtype) for a in out_avals
    ]
    out_arrs = sharded(*concat_in, *concat_zeros)
    return [
        {nm: np.asarray(out_arrs[i]).reshape(NCORES, *out_avals[i].shape)[c]
         for i, nm in enumerate(out_names)}
        for c in range(NCORES)
    ]


_BUILD_CACHE = {}


def _get_nc(steps, split, b3nz=True):
    key = (steps, split, b3nz)
    if key not in _BUILD_CACHE:
        _BUILD_CACHE[key] = _build(steps, split, b3nz)
    return _BUILD_CACHE[key]


def _prepare_in_maps(x, W1, b1, W2, b2, W3, b3, split):
    x = np.ascontiguousarray(np.asarray(x, np.float32))
    W1 = np.asarray(W1, np.float64)
    W2 = np.asarray(W2, np.float64)
    W3 = np.asarray(W3, np.float64)
    b1 = np.asarray(b1, np.float32)
    b2 = np.asarray(b2, np.float32)
    b3 = np.asarray(b3, np.float32)

    if split:
        w1h, w1l = _split_w(W1)
        w2h, w2l = _split_w(W2)
        w3h, w3l = _split_w(W3)
    else:
        w1h, w1l = _trunc12(W1), np.zeros((1, 1), np.float32)
        w2h, w2l = _trunc12(W2), np.zeros((1, 1), np.float32)
        w3h, w3l = _trunc12(W3), np.zeros((1, 1), np.float32)
        w1l = np.zeros_like(w1h)
        w2l = np.zeros_like(w2h)
        w3l = np.zeros_like(w3h)

    w3h_p = np.zeros((DH, W3P), np.float32)
    w3l_p = np.zeros((DH, W3P), np.float32)
    w3h_p[:, :1 + 2 * NB] = w3h
    w3l_p[:, :1 + 2 * NB] = w3l
    b3_p = np.zeros((1, W3P), np.float32)
    b3_p[0, :1 + 2 * NB] = b3

    common = {
        "w1xh": w1h[:DIN], "w1xl": w1l[:DIN],
        "w1sh": np.ascontiguousarray(w1h[DIN:]), "w1sl": np.ascontiguousarray(w1l[DIN:]),
        "w2h": w2h, "w2l": w2l, "w3h": w3h_p, "w3l": w3l_p,
        "b1c": np.ascontiguousarray(b1.reshape(KT2, 128).T),
        "b2c": np.ascontiguousarray(b2.reshape(KT2, 128).T),
        "b3row": b3_p,
        "ident": np.eye(128, dtype=np.float32),
    }
    in_maps = []
    for c in range(NCORES):
        m = dict(common)
        xT = np.ascontiguousarray(x[c * BC:(c + 1) * BC].T)
        xh = _trunc12(xT)
        m["xrh"] = xh
        m["xrl"] = _trunc12(
            (xT.astype(np.float64) - xh.astype(np.float64)).astype(np.float32))
        in_maps.append(m)
    return in_maps


def kernel(x, W1, b1, W2, b2, W3, b3):
    steps, split = STEPS, SPLIT
    in_maps = _prepare_in_maps(x, W1, b1, W2, b2, W3, b3, split)
    b3nz = bool(np.any(np.asarray(b3)))
    nc = _get_nc(steps, split, b3nz)
    results = _run_spmd(nc, in_maps)

    states = np.zeros((B, steps + 1, NB), np.float32)
    logpf = np.zeros((B, steps), np.float32)
    logpb = np.zeros((B, steps), np.float32)
    logpt = np.zeros((B, steps + 1), np.float32)
    for c in range(NCORES):
        r = results[c]
        rs = slice(c * BC, (c + 1) * BC)
        states[rs] = r["states_o"].reshape(BC, steps + 1, NB)
        logpf[rs] = r["logpf_o"]
        logpb[rs] = r["logpb_o"]
        logpt[rs] = r["logpt_o"]
    states[:, 0, :] = 0.0
    logpt[:, steps] = 0.0
    return states, logpf, logpb, logpt
